# revision 1
# baseline (speedup 1.0000x reference)
"""MoE AlltoAllTokenDispatcher kernel for TRN2 (8 NeuronCores).

The reference dispatcher's gather (tokens[argsort(idx)//k]) followed by
scatter-add at the same argsort permutation is an exact identity on slot
order: unpermuted[s] == tokens[s // k] for every slot s, independent of the
routing indices. The whole module therefore reduces to

    out[i] = tokens[i] * (probs[i, 0] + probs[i, 1])

which is a pure memory-bound row-scaling. We shard the token dimension
across the 8 cores (data-parallel, per the sharding hint's token-dim
sharding; no all-to-all is needed since expert compute is identity).
"""

import numpy as np

import concourse.bass as bass  # noqa: F401  (AP helpers)
import concourse.tile as tile
from concourse import bacc, mybir
from concourse.bass_utils import run_bass_kernel_spmd

N_TOKENS = 16384
HIDDEN = 4096
TOP_K = 2
N_CORES = 8
TOK_PER_CORE = N_TOKENS // N_CORES  # 2048
P = 128
N_TILES = TOK_PER_CORE // P  # 16

_nc_cache = None


def _build_nc():
    nc = bacc.Bacc(
        "TRN2",
        target_bir_lowering=False,
        debug=False,
        num_devices=N_CORES,
    )
    tokens = nc.dram_tensor(
        "tokens", [TOK_PER_CORE, HIDDEN], mybir.dt.float32, kind="ExternalInput"
    ).ap()
    probs = nc.dram_tensor(
        "probs", [TOK_PER_CORE, TOP_K], mybir.dt.float32, kind="ExternalInput"
    ).ap()
    out = nc.dram_tensor(
        "out", [TOK_PER_CORE, HIDDEN], mybir.dt.float32, kind="ExternalOutput"
    ).ap()

    with tile.TileContext(nc) as tc:
        with (
            tc.tile_pool(name="tok", bufs=4) as tok_pool,
            tc.tile_pool(name="pr", bufs=4) as pr_pool,
            tc.tile_pool(name="sc", bufs=4) as sc_pool,
        ):
            for i in range(N_TILES):
                tt = tok_pool.tile([P, HIDDEN], mybir.dt.float32)
                pt = pr_pool.tile([P, TOP_K], mybir.dt.float32)
                st = sc_pool.tile([P, 1], mybir.dt.float32)
                nc.sync.dma_start(out=tt[:], in_=tokens[i * P : (i + 1) * P, :])
                nc.sync.dma_start(out=pt[:], in_=probs[i * P : (i + 1) * P, :])
                nc.vector.reduce_sum(st[:], pt[:], axis=mybir.AxisListType.X)
                nc.vector.tensor_scalar_mul(tt[:], tt[:], st[:])
                nc.sync.dma_start(out=out[i * P : (i + 1) * P, :], in_=tt[:])
    nc.compile()
    return nc


def kernel(tokens, probs, indices=None, **_unused):
    global _nc_cache
    tokens = np.ascontiguousarray(np.asarray(tokens, dtype=np.float32))
    probs = np.ascontiguousarray(np.asarray(probs, dtype=np.float32))
    assert tokens.shape == (N_TOKENS, HIDDEN)
    assert probs.shape == (N_TOKENS, TOP_K)

    if _nc_cache is None:
        _nc_cache = _build_nc()
    nc = _nc_cache

    in_maps = [
        {
            "tokens": tokens[c * TOK_PER_CORE : (c + 1) * TOK_PER_CORE],
            "probs": probs[c * TOK_PER_CORE : (c + 1) * TOK_PER_CORE],
        }
        for c in range(N_CORES)
    ]
    res = run_bass_kernel_spmd(nc, in_maps, core_ids=list(range(N_CORES)))
    return np.concatenate([res.results[c]["out"] for c in range(N_CORES)], axis=0)


# revision 3
# speedup vs baseline: 1.2161x; 1.2161x over previous
"""MoE AlltoAllTokenDispatcher kernel for TRN2 (8 NeuronCores).

The reference dispatcher's gather (tokens[argsort(idx)//k]) followed by
scatter-add at the same argsort permutation is an exact identity on slot
order: unpermuted[s] == tokens[s // k] for every slot s, independent of the
routing indices. The whole module therefore reduces to

    out[i] = tokens[i] * (probs[i, 0] + probs[i, 1])

which is a pure memory-bound row-scaling. We shard the token dimension
across the 8 cores (data-parallel, per the sharding hint's token-dim
sharding; no all-to-all is needed since expert compute is identity).
"""

import numpy as np

import concourse.bass as bass  # noqa: F401  (AP helpers)
import concourse.tile as tile
from concourse import bacc, mybir
from concourse.bass_utils import run_bass_kernel_spmd

N_TOKENS = 16384
HIDDEN = 4096
TOP_K = 2
N_CORES = 8
TOK_PER_CORE = N_TOKENS // N_CORES  # 2048
P = 128
N_TILES = TOK_PER_CORE // P  # 16

_nc_cache = None


def _build_nc():
    nc = bacc.Bacc(
        "TRN2",
        target_bir_lowering=False,
        debug=False,
        num_devices=N_CORES,
    )
    tokens = nc.dram_tensor(
        "tokens", [TOK_PER_CORE, HIDDEN], mybir.dt.float32, kind="ExternalInput"
    ).ap()
    probs = nc.dram_tensor(
        "probs", [TOK_PER_CORE, TOP_K], mybir.dt.float32, kind="ExternalInput"
    ).ap()
    out = nc.dram_tensor(
        "out", [TOK_PER_CORE, HIDDEN], mybir.dt.float32, kind="ExternalOutput"
    ).ap()

    with tile.TileContext(nc) as tc:
        with (
            tc.tile_pool(name="tok", bufs=6) as tok_pool,
            tc.tile_pool(name="pr", bufs=1) as pr_pool,
            tc.tile_pool(name="sc", bufs=1) as sc_pool,
        ):
            # One gather DMA for all probs: partition p holds probs[n*128+p, :]
            # for all 16 tiles n -> [128, 32]; reduce pairs to scales [128, 16].
            pt = pr_pool.tile([P, N_TILES * TOP_K], mybir.dt.float32)
            st = sc_pool.tile([P, N_TILES], mybir.dt.float32)
            nc.sync.dma_start(
                out=pt[:].rearrange("p (n k) -> p n k", k=TOP_K),
                in_=probs.rearrange("(n p) k -> p n k", p=P),
            )
            nc.vector.reduce_sum(
                st[:],
                pt[:].rearrange("p (n k) -> p n k", k=TOP_K),
                axis=mybir.AxisListType.X,
            )

            # Loads on the sync HWDGE ring, stores on the scalar HWDGE ring so
            # a store waiting on compute never stalls a later load's dispatch.
            H2 = HIDDEN // 2
            for i in range(N_TILES):
                last = i == N_TILES - 1
                halves = 2 if last else 1
                for h in range(halves):
                    cols = slice(h * H2, HIDDEN if halves == 1 else (h + 1) * H2)
                    ncols = HIDDEN if halves == 1 else H2
                    tt = tok_pool.tile([P, ncols], mybir.dt.float32, tag="tok")
                    nc.sync.dma_start(
                        out=tt[:, :ncols],
                        in_=tokens[i * P : (i + 1) * P, cols],
                    )
                    nc.vector.tensor_scalar_mul(
                        tt[:, :ncols], tt[:, :ncols], st[:, i : i + 1]
                    )
                    nc.scalar.dma_start(
                        out=out[i * P : (i + 1) * P, cols], in_=tt[:, :ncols]
                    )
    nc.compile()
    return nc


def kernel(tokens, probs, indices=None, **_unused):
    global _nc_cache
    tokens = np.ascontiguousarray(np.asarray(tokens, dtype=np.float32))
    probs = np.ascontiguousarray(np.asarray(probs, dtype=np.float32))
    assert tokens.shape == (N_TOKENS, HIDDEN)
    assert probs.shape == (N_TOKENS, TOP_K)

    if _nc_cache is None:
        _nc_cache = _build_nc()
    nc = _nc_cache

    in_maps = [
        {
            "tokens": tokens[c * TOK_PER_CORE : (c + 1) * TOK_PER_CORE],
            "probs": probs[c * TOK_PER_CORE : (c + 1) * TOK_PER_CORE],
        }
        for c in range(N_CORES)
    ]
    res = run_bass_kernel_spmd(nc, in_maps, core_ids=list(range(N_CORES)))
    return np.concatenate([res.results[c]["out"] for c in range(N_CORES)], axis=0)
